# revision 36
# baseline (speedup 1.0000x reference)
"""MoE top-1 routing layer on 8 Trainium2 NeuronCores (expert-parallel).

Math: out[t] = (x[t] @ W[e] + b[e]) @ OW + ob   with e = argmax(x[t] @ GW + gb).

The layer is linear past the router, so the two matmuls fold into one:
  out[t] = x[t] @ M[e] + bias2[e],  M[e] = W[e] @ OW,  bias2[e] = b[e] @ OW + ob
M is precomputed on the host (one [D,H]@[H,O] sgemm per expert), cutting
device FLOPs 4x vs running the two matmuls on-core (D*O vs D*H + H*O per
token). The device then runs a single [C,D]@[D,O] GEMM per core.

Sharding: expert-parallel. Host computes the gate (fp64 -> exact argmax),
sorts tokens by expert, pads each expert's token set to capacity C, and
ships core k: xT (gathered tokens, transposed), M[k]. Each core returns
its C token outputs (bf16); host scatters rows back and adds bias2.
Tokens beyond capacity (never for balanced routing) fall back to a host
matmul.

Device inputs are host-packed into SBUF-stripe-major layouts so each DMA is
a single trigger with multi-KB contiguous descriptors:
  boot:  [128, D/128, XB0 + MG0]  (x token block 0 ++ M column chunk 0)
  xt{i}: [128, D/128, XB_i]       (remaining token blocks)
  m{i}:  [128, D/128, MG_i]       (remaining M column chunks)
The compute schedule interleaves (token-tile x column-chunk) units in DMA
arrival order so the PE starts after ~1MB of DMA and never waits again.
"""

import numpy as np
from contextlib import ExitStack

B, S, D, E, H, O = 4, 2048, 1024, 8, 2048, 1024
T = B * S
C = 1024          # per-expert token capacity (multiple of 128); tokens
                  # routed beyond capacity fall back to the host matmul
P = 128
KO_D = D // P     # 8

# "bf16": all matmul operands bf16 (fp32 PSUM accumulation) — fastest, rel
#         err ~5e-3.
MM_DT = "bf16"

XBLOCKS = [256, 256, 256, 256]    # x token DMA chunks (sum == C)
GCHUNKS = [128, 384, 512]         # M column DMA chunks (sum == O)
N_WARMUP = 10                     # leading dummy matmuls (PE p-state ramp)
assert sum(XBLOCKS) == C
assert sum(GCHUNKS) == O


def _legalize_waits(nc):
    """This container's walrus accepts 1 sem wait per instruction (2 for
    EventSemaphore); Tile's tail drain can carry more. Split the excess
    onto preceding same-engine NoOps."""
    from concourse import mybir

    uid = 0
    for f in nc.m.functions:
        for b in f.blocks:
            insts = b.instructions
            out = []
            changed = False
            for ins in insts:
                si = ins.sync_info
                waits = list(si.on_wait) if si is not None else []
                limit = 2 if str(ins.opcode) == "EventSemaphore" else 1
                if len(waits) > limit:
                    extra, keep = waits[:-limit], waits[-limit:]
                    for w in extra:
                        uid += 1
                        out.append(
                            mybir.InstNoOp(
                                name=f"waitsplit-{uid}",
                                engine=ins.engine,
                                sync_info=mybir.SyncInfo(on_wait=[w], on_update=[]),
                                bass_nofuse=True,
                            )
                        )
                    si.on_wait = keep
                    changed = True
                out.append(ins)
            if changed:
                insts.clear()
                insts.extend(out)


def _patch_tail_barrier(tile_mod):
    """Tile's kernel tail is drain -> barrier -> sem-reset -> barrier.
    The second all-engine barrier only orders the sem-reset against program
    end, which the per-engine stream end already guarantees; drop it."""
    if getattr(tile_mod.TileContext, "_moe_tail_patched", False):
        return
    from concourse.vector_clock import ScopedClock

    def _drain_and_barrier(self, tick_clock, wait_clock):
        drain_inst = self.nc.sync.drain()
        wait_clock.add_sem_waits(
            drain_inst.ins, ScopedClock({None: tick_clock.global_clock})
        )
        self.nc.all_engine_barrier()
        popped = self.nc._tile_sem_poison_stack.pop()
        assert popped is self._sem_poison
        self.nc.clear_and_free_semaphores(list(self.sems.allocated().values()))

    tile_mod.TileContext._drain_and_barrier = _drain_and_barrier
    tile_mod.TileContext._moe_tail_patched = True


def _emit(nc, tile, mm_dt, f32):
    """Single fused GEMM out[C,O] = xT^T @ M, streamed in arrival order.

    Loop order is k-inner with both O-chunks interleaved so consecutive
    matmuls share the stationary x tile (walrus can then reuse the loaded
    weights instead of re-streaming LDWEIGHTS every matmul). Warm-up
    matmuls on a scratch tile run during the boot DMA so the PE p-state
    ramp completes before real work arrives.
    """
    XB0 = XBLOCKS[0]
    MGA, MGB, MGC = GCHUNKS
    # boot = first x chunk ++ first (narrow) M column chunk: the minimum
    # data needed to start real compute
    boot = nc.dram_tensor("boot", [P, KO_D, XB0 + MGA], mm_dt,
                          kind="ExternalInput")
    mgb = nc.dram_tensor("mgb", [P, KO_D, MGB], mm_dt, kind="ExternalInput")
    xc1 = nc.dram_tensor("xc1", [P, KO_D, XBLOCKS[1]], mm_dt,
                         kind="ExternalInput")
    mgc = nc.dram_tensor("mgc", [P, KO_D, MGC], mm_dt, kind="ExternalInput")
    xc2 = nc.dram_tensor("xc2", [P, KO_D, XBLOCKS[2]], mm_dt,
                         kind="ExternalInput")
    xc3 = nc.dram_tensor("xc3", [P, KO_D, XBLOCKS[3]], mm_dt,
                         kind="ExternalInput")
    out = nc.dram_tensor("out", [C, O], mm_dt, kind="ExternalOutput")

    with tile.TileContext(nc) as tc:
        with ExitStack() as ctx:
            x_pool = ctx.enter_context(tc.tile_pool(name="x", bufs=1))
            m_pool = ctx.enter_context(tc.tile_pool(name="m", bufs=1))
            st_pool = ctx.enter_context(tc.tile_pool(name="st", bufs=4))
            ps_pool = ctx.enter_context(
                tc.tile_pool(name="ps", bufs=3, space="PSUM")
            )
            wps_pool = ctx.enter_context(
                tc.tile_pool(name="wps", bufs=1, space="PSUM")
            )

            boot_sb = x_pool.tile([P, KO_D, XB0 + MGA], mm_dt)
            x_sbs = [
                x_pool.tile([P, KO_D, bw], mm_dt, name=f"x{i}")
                for i, bw in enumerate(XBLOCKS)
                if i > 0
            ]
            mgb_sb = m_pool.tile([P, KO_D, MGB], mm_dt, name="mb")
            mgc_sb = m_pool.tile([P, KO_D, MGC], mm_dt, name="mc")
            dummy = x_pool.tile([P, 512], mm_dt, name="dummy")

            # demand-ordered loads, all on the sync queue (the Activation
            # queue is reserved for output traffic)
            nc.sync.dma_start(boot_sb[:], boot[:])
            nc.sync.dma_start(mgb_sb[:], mgb[:])
            nc.sync.dma_start(x_sbs[0][:], xc1[:])
            nc.sync.dma_start(mgc_sb[:], mgc[:])
            nc.sync.dma_start(x_sbs[1][:], xc2[:])
            nc.sync.dma_start(x_sbs[2][:], xc3[:])

            nc.gpsimd.memset(dummy[:], 0)
            wps = wps_pool.tile([P, 512], f32, name="wps")

            def warm(n):
                """dummy matmuls: p-state ramp + gap filler at tight gates"""
                for _ in range(n):
                    nc.tensor.matmul(wps, dummy[:, :P], dummy[:], start=True,
                                     stop=True)

            warm(N_WARMUP)

            # stationary x tile for (token-tile t, contraction k)
            def x_src(t, k):
                c = t * P
                if c < XB0:
                    return boot_sb[:, k, c : c + P]
                i = (c - XB0) // 256
                c = (c - XB0) % 256
                return x_sbs[i][:, k, c : c + P]

            def m_src(g):
                if g == 0:
                    return boot_sb[:, :, XB0 : XB0 + MGA]
                return mgb_sb if g == 1 else mgc_sb

            G0 = [0, MGA, MGA + MGB]
            sts = {}

            def unit(t, g):
                """token tile t x O-chunk g; one out-DMA per (tile, row-half)
                to keep the DMA count low and the final chain short"""
                gw = GCHUNKS[g]
                msb = m_src(g)
                ps = ps_pool.tile([P, 512], f32, name="ps")[:, :gw]
                for k in range(KO_D):
                    nc.tensor.matmul(
                        ps,
                        x_src(t, k),
                        msb[:, k, :],
                        start=(k == 0),
                        stop=(k == KO_D - 1),
                    )
                if g == 0:
                    sts[t] = st_pool.tile([P, O], mm_dt, name="st")
                st = sts[t]
                c0 = G0[g]
                r0 = t * P
                if g == 0:
                    nc.vector.tensor_copy(st[:, c0 : c0 + gw], ps)
                    return
                if g == 1:
                    # first half of the row is complete: ship it now so the
                    # final post-stream DMA chain is half as long
                    nc.vector.tensor_copy(st[:, c0 : c0 + gw], ps)
                    nc.scalar.dma_start(out[r0 : r0 + P, :c0 + gw],
                                        st[:, : c0 + gw])
                    return
                nc.vector.tensor_copy(st[:, c0:], ps)
                nc.scalar.dma_start(out[r0 : r0 + P, c0:], st[:, c0:])

            def unit_tail(t):
                """final tile's wide chunk as two half-width sub-units: the
                second half's matmuls overlap the first half's cast+DMA, so
                the post-stream chain is half as long"""
                msb = m_src(2)
                st = sts[t]
                c0 = G0[2]
                r0 = t * P
                h = GCHUNKS[2] // 2
                for i, q in enumerate((nc.scalar, nc.sync)):
                    ps = ps_pool.tile([P, 512], f32, name="ps")[:, :h]
                    for k in range(KO_D):
                        nc.tensor.matmul(
                            ps,
                            x_src(t, k),
                            msb[:, k, i * h : (i + 1) * h],
                            start=(k == 0),
                            stop=(k == KO_D - 1),
                        )
                    nc.vector.tensor_copy(st[:, c0 + i * h : c0 + (i + 1) * h], ps)
                    q.dma_start(
                        out[r0 : r0 + P, c0 + i * h : c0 + (i + 1) * h],
                        st[:, c0 + i * h : c0 + (i + 1) * h],
                    )

            # demand-ordered schedule (chunks gate in DMA arrival order)
            unit(0, 0); unit(1, 0)              # boot: x0 + mg_a
            unit(0, 1); unit(1, 1)              # mgb
            unit(2, 0); unit(3, 0)              # xc1
            unit(2, 1); unit(3, 1)
            unit(0, 2); unit(1, 2)              # mgc
            unit(2, 2); unit(3, 2)
            unit(4, 0); unit(5, 0)              # xc2
            unit(4, 1); unit(5, 1)
            unit(4, 2); unit(5, 2)
            unit(6, 0); unit(7, 0)              # xc3
            unit(6, 1); unit(7, 1)
            unit(6, 2)
            unit_tail(7)
    return nc


def _patch_walrus_policy():
    """Compile with walrus --policy=2 (heuristics post-scheduler): measured
    ~1.5us faster than the default --policy=0 on this kernel."""
    import concourse.bass_utils as bu

    if getattr(bu, "_moe_policy_patched", False):
        return
    orig = bu.run_command

    def _rc(argv, **kw):
        if argv and "walrus_driver" in str(argv[0]):
            argv = ["--policy=2" if a == "--policy=0" else a for a in argv]
        return orig(argv, **kw)

    bu.run_command = _rc
    bu._moe_policy_patched = True


def _build_nc():
    import concourse.bass as bass
    import concourse.tile as tile
    from concourse import mybir

    _patch_tail_barrier(tile)
    _patch_walrus_policy()
    f32 = mybir.dt.float32
    mm_dt = mybir.dt.bfloat16 if MM_DT == "bf16" else mybir.dt.float32r
    nc = bass.Bass()
    _emit(nc, tile, mm_dt, f32)
    _legalize_waits(nc)
    return nc


_NC_CACHE = {}


def kernel(x, gate_w, gate_b, expert_w, expert_b, out_w, out_b):
    import os

    # The device path runs through the axon PJRT plugin; make sure a
    # harness-pinned JAX_PLATFORMS=cpu doesn't exclude it.
    plats = os.environ.get("JAX_PLATFORMS")
    if plats and "axon" not in plats:
        os.environ["JAX_PLATFORMS"] = plats + ",axon"

    from concourse.bass_utils import run_bass_kernel_spmd

    x = np.asarray(x, dtype=np.float32)
    gate_w = np.asarray(gate_w, dtype=np.float32)
    gate_b = np.asarray(gate_b, dtype=np.float32)
    expert_w = np.asarray(expert_w, dtype=np.float32)
    expert_b = np.asarray(expert_b, dtype=np.float32)
    out_w = np.asarray(out_w, dtype=np.float32)
    out_b = np.asarray(out_b, dtype=np.float32)

    xt = x.reshape(T, D)
    # Gate on host in fp64: argmax matches the fp32 reference exactly
    # (min top-2 logit gap is ~1e-5, fp64 error ~1e-12).
    logits = xt.astype(np.float64) @ gate_w.astype(np.float64) + gate_b.astype(
        np.float64
    )
    idx = np.argmax(logits, axis=1)

    import ml_dtypes

    mm_np = ml_dtypes.bfloat16

    # Fold the two device matmuls into one: M[e] = W[e] @ OW  (fp32 sgemm)
    M_all = np.matmul(expert_w, out_w)  # [E, D, O]

    tok_of_expert = [np.nonzero(idx == e)[0] for e in range(E)]
    in_maps = []
    kept = []
    overflow = []
    for e in range(E):
        toks = tok_of_expert[e]
        if len(toks) > C:
            overflow.append((e, toks[C:]))
            toks = toks[:C]
        kept.append(toks)
        xpad = np.zeros((D, C), dtype=mm_np)
        xpad[:, : len(toks)] = xt[toks].T.astype(mm_np)
        # stripe-major: xk[k, p, c] = xpad[k*128+p, c]
        xk = xpad.reshape(KO_D, P, C)
        # m packed [P, KO_D, O]: m[p, k, j] = M[k*128+p, j]
        mp = M_all[e].astype(mm_np).reshape(KO_D, P, O).transpose(1, 0, 2)
        XB0 = XBLOCKS[0]
        MGA, MGB, MGC = GCHUNKS
        x0p = xk[:, :, :XB0].transpose(1, 0, 2)
        im = {
            "boot": np.ascontiguousarray(
                np.concatenate([x0p, mp[:, :, :MGA]], axis=2)
            ),
            "mgb": np.ascontiguousarray(mp[:, :, MGA : MGA + MGB]),
            "mgc": np.ascontiguousarray(mp[:, :, MGA + MGB :]),
        }
        c0 = XB0
        for i, bw in enumerate(XBLOCKS[1:], start=1):
            im[f"xc{i}"] = np.ascontiguousarray(
                xk[:, :, c0 : c0 + bw].transpose(1, 0, 2)
            )
            c0 += bw
        in_maps.append(im)

    if "nc" not in _NC_CACHE:
        _NC_CACHE["nc"] = _build_nc()
    nc = _NC_CACHE["nc"]

    res = run_bass_kernel_spmd(nc, in_maps, list(range(E)))

    bias2 = (
        expert_b.astype(np.float64) @ out_w.astype(np.float64)
        + out_b.astype(np.float64)
    ).astype(np.float32)  # [E, O]

    out = np.empty((T, O), dtype=np.float32)
    for e in range(E):
        toks = kept[e]
        out[toks] = res.results[e]["out"][: len(toks)].astype(np.float32) + bias2[e]
    for e, toks in overflow:
        out[toks] = (xt[toks] @ M_all[e]) + bias2[e]
    return out.reshape(B, S, O)
